# revision 45
# baseline (speedup 1.0000x reference)
"""Multi-head causal attention (B=4,T=2048,C=1024,H=16,HS=64) on 8 TRN2 cores.

Sharding: core c -> batch b=c//2, head-group hg=c%2 (8 heads each).
Each core computes QKV projections for its heads, causal flash-attention,
and a partial output projection over its 512 combo channels, emitting
out^T partial [1024, 2048].  Host sums the two partials per batch (the
tensor-parallel all-reduce) and transposes.

Matmuls run in bf16 (fp32 PSUM accumulation); softmax skips
max-subtraction (scores ~ N(0,1), exp never overflows); the softmax
denominator comes free as a 65th row of the PV matmul via a ones-column
appended to V.

v2 restructure vs the 350us baseline:
  - Heads are processed in PAIRS: the two 64-contraction score matmuls
    of a pair go to disjoint PE row groups (tile_position (0,0)/(64,0))
    back-to-back, so they stream concurrently -> scores at ~full PE rate
    instead of half.
  - po is one [128, 2, 512] psum tile (bank j = head j of the pair), so
    each head keeps its own clean accumulation group.
  - The attention phase is exp(ACT)-bound; QKV projection of block tb+1
    and the output projection are interleaved between head-pairs so the
    PE fills those gaps with projection matmuls.
  - exp instructions skip the fully-masked prefix of diagonal chunks.
  - po is freed early by a single [65,2,512] psum->sbuf copy; the
    normalization chain (recip, broadcast, muls) runs off SBUF.
"""

import os
import sys

if "/opt/trn_rl_repo" not in sys.path:
    sys.path.insert(0, "/opt/trn_rl_repo")

import ml_dtypes
import numpy as np

import concourse.mybir as mybir
import concourse.tile as tile
from concourse import bacc
from concourse.bass_utils import run_bass_kernel_spmd

P = 128
B, T, C, H = 4, 2048, 1024, 16
HS = C // H              # 64
HL = H // 2              # 8 local heads per core
HD = HL * HS             # 512 local combo channels
NT = T // 512            # 4 query blocks of 512
NCC = C // P             # 8 contraction chunks over C
NKC = T // P             # 16 key chunks of 128
NHP = HL // 2            # 4 local head pairs
F32 = mybir.dt.float32
BF16 = mybir.dt.bfloat16
EXP_SCALE = float(HS) ** -0.5  # 1/8, folded into the exp activation

MM_DT = BF16
MM_NP = ml_dtypes.bfloat16

_PROGRAM = None


def _build_program():
    nc = bacc.Bacc("TRN2", target_bir_lowering=False, debug=False, num_devices=8)

    xT = nc.dram_tensor("xT", [C, T], MM_DT, kind="ExternalInput")
    wq = nc.dram_tensor("wq", [C, HD], MM_DT, kind="ExternalInput")
    wk = nc.dram_tensor("wk", [C, HD], MM_DT, kind="ExternalInput")
    wv = nc.dram_tensor("wv", [C, HD], MM_DT, kind="ExternalInput")
    wpT = nc.dram_tensor("wpT", [HD, C], MM_DT, kind="ExternalInput")
    bias = nc.dram_tensor("bias", [C], F32, kind="ExternalInput")
    masks = nc.dram_tensor("masks", [P, P], MM_DT, kind="ExternalInput")
    # bf16 output halves the output-DMA tail; the host all-reduce of the
    # two per-batch partials runs in fp32
    outT = nc.dram_tensor("outT", [C, T], MM_DT, kind="ExternalOutput")

    Exp = mybir.ActivationFunctionType.Exp

    with tile.TileContext(nc) as tc:
        with (
            tc.tile_pool(name="persist", bufs=1) as persist,
            tc.tile_pool(name="xtp", bufs=2) as xtp,
            tc.tile_pool(name="ptp", bufs=8) as ptp,
            tc.tile_pool(name="rawp", bufs=3) as rawp,
            tc.tile_pool(name="misc", bufs=2) as misc,
            tc.tile_pool(name="outp", bufs=3) as outp,
            tc.tile_pool(name="ps_s", bufs=2, space="PSUM") as ps_s,
            tc.tile_pool(name="ps_o", bufs=1, space="PSUM") as ps_o,
            tc.tile_pool(name="ps_gen", bufs=2, space="PSUM") as ps_gen,
        ):
            # Q^T / K^T with head pairs stacked on partitions: tile hp holds
            # head 2hp in rows 0-63 and head 2hp+1 in rows 64-127.  Split
            # per head-pair (and vaug per block) so the dependency tracker
            # sees exact producer/consumer pairs instead of one big tensor.
            qts = [persist.tile([P, T], MM_DT, tag=f"qt{i}", name=f"qt{i}") for i in range(NHP)]
            kts = [persist.tile([P, T], MM_DT, tag=f"kt{i}", name=f"kt{i}") for i in range(NHP)]
            vaugs = [
                persist.tile([P, 4, HL, HS + 1], MM_DT, tag=f"vaug{i}", name=f"vaug{i}")
                for i in range(NT)
            ]
            bias_sb = persist.tile([P, C // P], F32)
            wq_sb = persist.tile([P, NCC, HD], MM_DT, tag="wq")
            wk_sb = persist.tile([P, NCC, HD], MM_DT, tag="wk")
            wv_sb = persist.tile([P, NCC, HD], MM_DT, tag="wv")
            wpT_sb = persist.tile([P, HD // P, C], MM_DT, tag="wpT")
            masks_sb = persist.tile([P, 2, P], MM_DT, tag="masks")
            combos = [persist.tile([P, T], MM_DT, tag=f"combo{i}", name=f"combo{i}") for i in range(NHP)]
            ones1 = persist.tile([1, HS], F32, tag="ones1")

            # ones column for the softmax-denominator row of the PV matmul
            for vv in vaugs:
                nc.vector.memset(vv[:, :, :, HS : HS + 1], 1.0)
            nc.vector.memset(ones1[:], 1.0)
            # DMA priority follows first consumption: q-chain 0 (wq + x^T
            # block 0), then the v chains (wv), then k chains (wk);
            # wpT/masks/bias are needed much later.
            xt_tiles = {}
            xt0 = xtp.tile([P, NCC, 512], MM_DT, tag="xt")
            xt_tiles[0] = xt0
            xTr = xT[:].rearrange("(co p) t -> p co t", p=P)
            wqr = wq[:].rearrange("(co p) n -> p co n", p=P)
            wkr = wk[:].rearrange("(co p) n -> p co n", p=P)
            wvr = wv[:].rearrange("(co p) n -> p co n", p=P)
            for _h in range(4):
                nc.sync.dma_start(xt0[:, 2 * _h : 2 * _h + 2, :], xTr[:, 2 * _h : 2 * _h + 2, 0:512])
            for _h in range(4):
                nc.scalar.dma_start(wq_sb[:, 2 * _h : 2 * _h + 2, :], wqr[:, 2 * _h : 2 * _h + 2, :])
            for _h in range(4):
                nc.scalar.dma_start(wv_sb[:, 2 * _h : 2 * _h + 2, :], wvr[:, 2 * _h : 2 * _h + 2, :])
            for _h in range(4):
                nc.sync.dma_start(wk_sb[:, 2 * _h : 2 * _h + 2, :], wkr[:, 2 * _h : 2 * _h + 2, :])
            nc.gpsimd.dma_start(masks_sb[:, 0, :], masks[:])
            nc.gpsimd.dma_start(masks_sb[:, 1, :], masks[:])
            nc.gpsimd.dma_start(bias_sb[:], bias[:].rearrange("(db p) -> p db", p=P))

            def qk_quarter(tb, part):
                """Q and K projection chains for head-pair `part` of
                block tb."""
                tsl = slice(tb * 512, (tb + 1) * 512)
                xt = xt_tiles[tb]
                hsl = slice(part * P, (part + 1) * P)
                for w_sb, dst in ((wq_sb, qts[part]), (wk_sb, kts[part])):
                    pqk = ps_gen.tile([P, 512], F32, tag="pqk")
                    for co in range(NCC):
                        nc.tensor.matmul(
                            pqk[:],
                            w_sb[:, co, hsl],
                            xt[:, co, :],
                            start=(co == 0),
                            stop=(co == NCC - 1),
                        )
                    if tb == 0:
                        # ACT is exp-idle before attention starts; use it
                        # for block 0's psum evacuation
                        nc.scalar.copy(out=dst[:, tsl], in_=pqk[:])
                    else:
                        nc.vector.tensor_copy(out=dst[:, tsl], in_=pqk[:])

            def v_quarter(tb, part):
                """V projection chain for token sub-block `part` of block
                tb.  part=0 also issues the x^T block load."""
                tsl = slice(tb * 512, (tb + 1) * 512)
                if part == 0 and tb not in xt_tiles:
                    xt = xtp.tile([P, NCC, 512], MM_DT, tag="xt")
                    xt_tiles[tb] = xt
                    for _h in range(2):
                        nc.sync.dma_start(
                            xt[:, 4 * _h : 4 * _h + 4, :],
                            xTr[:, 4 * _h : 4 * _h + 4, tsl],
                        )
                xt = xt_tiles[tb]
                pv = ps_gen.tile([P, 512], F32, tag="pqk")
                for co in range(NCC):
                    nc.tensor.matmul(
                        pv[:],
                        xt[:, co, part * P : (part + 1) * P],
                        wv_sb[:, co, :],
                        start=(co == 0),
                        stop=(co == NCC - 1),
                    )
                nc.vector.tensor_copy(
                    out=vaugs[tb][:, part, :, 0:HS],
                    in_=pv[:].rearrange("p (h d) -> p h d", h=HL),
                )

            def attn_pair(qb, hp, fast_tail=False):
                """Causal attention for head pair hp over query block qb."""
                q0 = qb * 512
                nkc = 4 * (qb + 1)
                po = ps_o.tile([P, 2, 512], F32, tag="po")
                for kc in range(nkc):
                    mi = kc - 4 * qb
                    # diagonal chunks: columns q < kc*128-q0 are fully
                    # masked; skip them in scores, exp and PV
                    c0 = kc * P - q0 if mi >= 0 else 0
                    pss = ps_s.tile([P, 2, 512], F32, tag="pss")
                    for j in range(2):
                        r0 = 64 * j
                        nc.tensor.matmul(
                            pss[:, j, c0:512],
                            kts[hp][r0 : r0 + 64, kc * P : (kc + 1) * P],
                            qts[hp][r0 : r0 + 64, q0 + c0 : q0 + 512],
                            start=True,
                            stop=True,
                            tile_position=(r0, 0),
                        )
                    pt = ptp.tile([P, 2, 512], MM_DT, tag="pt")
                    nc.scalar.activation(
                        pt[:, :, c0:512], pss[:, :, c0:512], Exp, scale=EXP_SCALE
                    )
                    if mi >= 0:
                        # only the 128x128 diagonal block needs the
                        # triangle mask (one mul covers both heads)
                        msl = slice(mi * P, (mi + 1) * P)
                        nc.vector.tensor_mul(
                            out=pt[:, :, msl],
                            in0=pt[:, :, msl],
                            in1=masks_sb[:],
                        )
                    for j in range(2):
                        nc.tensor.matmul(
                            po[0 : HS + 1, j, c0:512],
                            vaugs[kc // 4][:, kc % 4, 2 * hp + j, :],
                            pt[:, j, c0:512],
                            start=(kc == 0),
                            stop=(kc == nkc - 1),
                        )
                # single psum->sbuf copy frees po for the next pair FIRST
                # (the next pair's PV needs the banks ~1.4us in); the
                # normalization chain then runs off SBUF
                if fast_tail:
                    # latency-critical final pair: no next pair needs the
                    # po banks, so skip the raw staging copy and run the
                    # chain per head straight off PSUM ([1,N] DVE ops are
                    # lane-serial: ~0.6us for N=512)
                    for j in range(2):
                        dj = misc.tile([1, 2, 512], F32, tag="den2")
                        nc.vector.tensor_copy(out=dj[0:1, 0, :], in_=po[HS : HS + 1, j, :])
                        rj = misc.tile([1, 2, 512], F32, tag="rc")
                        nc.vector.reciprocal_approx_fast(rj[0:1, 0, :], dj[0:1, 0, :])
                        rbj = misc.tile([HS, 2, 512], F32, tag="rb")
                        nc.gpsimd.partition_broadcast(rbj[:, 0, :], rj[0:1, 0, :])
                        nc.vector.tensor_mul(
                            out=combos[hp][j * HS : (j + 1) * HS, q0 : q0 + 512],
                            in0=po[0:HS, j, :],
                            in1=rbj[:, 0, :],
                        )
                else:
                    raw = rawp.tile([HS + 1, 2, 512], MM_DT, tag="raw")
                    nc.vector.tensor_copy(out=raw[:], in_=po[0 : HS + 1, :, :])
                    den2 = misc.tile([1, 2, 512], F32, tag="den2")
                    nc.vector.tensor_copy(out=den2[:], in_=raw[HS : HS + 1, :, :])
                    rc = misc.tile([1, 2, 512], F32, tag="rc")
                    nc.vector.reciprocal_approx_fast(rc[:], den2[:])
                    rb = misc.tile([HS, 2, 512], F32, tag="rb")
                    nc.gpsimd.partition_broadcast(rb[:], rc[:])
                    nc.vector.tensor_mul(
                        out=combos[hp][0:HS, q0 : q0 + 512],
                        in0=raw[0:HS, 0, :],
                        in1=rb[:, 0, :],
                    )
                    nc.vector.tensor_mul(
                        out=combos[hp][HS:P, q0 : q0 + 512],
                        in0=raw[0:HS, 1, :],
                        in1=rb[:, 1, :],
                    )

            def outproj(qb, dbs=None):
                q0 = qb * 512
                for db in dbs if dbs is not None else range(C // P):
                    pp = ps_gen.tile([P, 512], F32, tag="pqk")
                    for co in range(HD // P):
                        nc.tensor.matmul(
                            pp[:],
                            wpT_sb[:, co, db * P : (db + 1) * P],
                            combos[co][:, q0 : q0 + 512],
                            start=(co == 0),
                            stop=(co == HD // P - 1),
                        )
                    ot = outp.tile([P, 512], MM_DT, tag="ot")
                    nc.vector.tensor_scalar_add(ot[:], pp[:], bias_sb[:, db : db + 1])
                    eng = nc.sync if db % 2 == 0 else nc.gpsimd
                    eng.dma_start(outT[db * P : (db + 1) * P, q0 : q0 + 512], ot[:])

            # ---- schedule ----
            # The input DMA paces the first ~13us; dummy matmuls on a
            # zeroed SBUF tile keep the PE HAM activity monitor busy so
            # the clock gate is at 8/8 (2.4 GHz) when real matmuls start
            # (otherwise all of QKV(0) runs at the cold 1.2 GHz).
            dummy = persist.tile([P, P], MM_DT, tag="dummy")
            nc.vector.memset(dummy[:], 0.0)
            # trigger the ~2.7us exp ACT-table load during the DMA head
            # instead of at the first real softmax (write to scratch so
            # the warm-up matmuls below don't serialize behind it)
            nc.scalar.activation(
                ones1[0:1, 0:1], dummy[0:1, 0:1], Exp, scale=EXP_SCALE
            )
            pw = ps_gen.tile([P, P], F32, tag="pqk")
            for _w in range(70):
                nc.tensor.matmul(pw[:, 0:64], dummy[:], dummy[:, 0:64], start=True, stop=True)
            # attn(qb, hp) needs: q/k chains (tb<=qb, part=hp) and ALL v
            # chains of blocks <= qb.  Block 0's chains cascade so
            # attention starts as soon as [qk(0,0), v(0,0..3)] land; later
            # blocks' projections sit a full window ahead as exp-gap
            # filler.  qb 3 has no QKV left, so the output projections of
            # blocks 0-2 are delayed into it.
            qk_quarter(0, 0)
            for part in range(4):
                v_quarter(0, part)
            for qb in range(NT):
                for hp in range(NHP):
                    attn_pair(qb, hp, fast_tail=(qb == NT - 1 and hp == NHP - 1))
                    if qb == 0 and hp < 3:
                        qk_quarter(0, hp + 1)
                    if qb == 0 and hp == 3:
                        # wpT is first needed by outproj during the last
                        # window; loading it now keeps it clear of the
                        # startup DMA burst
                        nc.gpsimd.dma_start(wpT_sb[:], wpT[:].rearrange("(co p) n -> p co n", p=P))
                    if qb < NT - 1:
                        v_quarter(qb + 1, hp)
                        qk_quarter(qb + 1, hp)
                    elif hp < 3:
                        outproj(hp, dbs=range(0, 4))
            # the remaining hp3-independent outproj chains flow into the
            # last pair's exp gaps and its normalization-chain latency
            # (keeping the PE warm); outproj(3) runs immediately after
            for qq in range(3):
                outproj(qq, dbs=range(4, 8))
            outproj(NT - 1)

    nc.finalize()
    return nc


def _causal_masks():
    # lower-triangle [128,128]: 1.0 iff kl <= ql (applied multiplicatively
    # post-exp to the single diagonal block of each diagonal key chunk)
    kl = np.arange(P)[:, None]
    ql = np.arange(P)[None, :]
    return (kl <= ql).astype(np.float32)


def _in_maps(x, Wq, Wk, Wv, Wproj, bproj):
    masks = _causal_masks()
    zeros_bias = np.zeros_like(bproj)
    maps = []
    for core in range(8):
        b, hg = core // 2, core % 2
        hs = slice(hg * HL, (hg + 1) * HL)
        maps.append(
            {
                "xT": np.ascontiguousarray(x[b].T).astype(MM_NP),
                "wq": np.ascontiguousarray(
                    Wq[hs].transpose(1, 0, 2).reshape(C, HD).astype(MM_NP)
                ),
                "wk": np.ascontiguousarray(
                    Wk[hs].transpose(1, 0, 2).reshape(C, HD).astype(MM_NP)
                ),
                "wv": np.ascontiguousarray(
                    Wv[hs].transpose(1, 0, 2).reshape(C, HD).astype(MM_NP)
                ),
                "wpT": np.ascontiguousarray(Wproj[:, hg * HD : (hg + 1) * HD].T).astype(MM_NP),
                "bias": np.ascontiguousarray(bproj if hg == 0 else zeros_bias),
                "masks": masks.astype(MM_NP),
            }
        )
    return maps


def get_program():
    global _PROGRAM
    if _PROGRAM is None:
        _PROGRAM = _build_program()
    return _PROGRAM


def kernel(x, Wq, Wk, Wv, Wproj, bproj, _run_kwargs=None):
    x = np.asarray(x, dtype=np.float32)
    Wq = np.asarray(Wq, dtype=np.float32)
    Wk = np.asarray(Wk, dtype=np.float32)
    Wv = np.asarray(Wv, dtype=np.float32)
    Wproj = np.asarray(Wproj, dtype=np.float32)
    bproj = np.asarray(bproj, dtype=np.float32)

    nc = get_program()
    res = run_bass_kernel_spmd(
        nc,
        _in_maps(x, Wq, Wk, Wv, Wproj, bproj),
        core_ids=list(range(8)),
        **(_run_kwargs or {}),
    )
    out = np.empty((B, T, C), dtype=np.float32)
    for b in range(B):
        out[b] = (
            res.results[2 * b]["outT"].astype(np.float32)
            + res.results[2 * b + 1]["outT"].astype(np.float32)
        ).T
    kernel.last_results = res
    return out


# revision 50
# speedup vs baseline: 1.0061x; 1.0061x over previous
"""Multi-head causal attention (B=4,T=2048,C=1024,H=16,HS=64) on 8 TRN2 cores.

Sharding: core c -> batch b=c//2, head-group hg=c%2 (8 heads each).
Each core computes QKV projections for its heads, causal flash-attention,
and a partial output projection over its 512 combo channels, emitting
out^T partial [1024, 2048].  Host sums the two partials per batch (the
tensor-parallel all-reduce) and transposes.

Matmuls run in bf16 (fp32 PSUM accumulation); softmax skips
max-subtraction (scores ~ N(0,1), exp never overflows); the softmax
denominator comes free as a 65th row of the PV matmul via a ones-column
appended to V.

v2 restructure vs the 350us baseline:
  - Heads are processed in PAIRS: the two 64-contraction score matmuls
    of a pair go to disjoint PE row groups (tile_position (0,0)/(64,0))
    back-to-back, so they stream concurrently -> scores at ~full PE rate
    instead of half.
  - po is one [128, 2, 512] psum tile (bank j = head j of the pair), so
    each head keeps its own clean accumulation group.
  - The attention phase is exp(ACT)-bound; QKV projection of block tb+1
    and the output projection are interleaved between head-pairs so the
    PE fills those gaps with projection matmuls.
  - exp instructions skip the fully-masked prefix of diagonal chunks.
  - po is freed early by a single [65,2,512] psum->sbuf copy; the
    normalization chain (recip, broadcast, muls) runs off SBUF.
"""

import os
import sys

if "/opt/trn_rl_repo" not in sys.path:
    sys.path.insert(0, "/opt/trn_rl_repo")

import ml_dtypes
import numpy as np

import concourse.mybir as mybir
import concourse.tile as tile
from concourse import bacc
from concourse.bass_utils import run_bass_kernel_spmd

P = 128
B, T, C, H = 4, 2048, 1024, 16
HS = C // H              # 64
HL = H // 2              # 8 local heads per core
HD = HL * HS             # 512 local combo channels
NT = T // 512            # 4 query blocks of 512
NCC = C // P             # 8 contraction chunks over C
NKC = T // P             # 16 key chunks of 128
NHP = HL // 2            # 4 local head pairs
F32 = mybir.dt.float32
BF16 = mybir.dt.bfloat16
EXP_SCALE = float(HS) ** -0.5  # 1/8, folded into the exp activation

MM_DT = BF16
MM_NP = ml_dtypes.bfloat16

_PROGRAM = None


def _build_program():
    nc = bacc.Bacc("TRN2", target_bir_lowering=False, debug=False, num_devices=8)

    xT = nc.dram_tensor("xT", [C, T], MM_DT, kind="ExternalInput")
    wq = nc.dram_tensor("wq", [C, HD], MM_DT, kind="ExternalInput")
    wk = nc.dram_tensor("wk", [C, HD], MM_DT, kind="ExternalInput")
    wv = nc.dram_tensor("wv", [C, HD], MM_DT, kind="ExternalInput")
    wpT = nc.dram_tensor("wpT", [HD, C], MM_DT, kind="ExternalInput")
    bias = nc.dram_tensor("bias", [C], F32, kind="ExternalInput")
    masks = nc.dram_tensor("masks", [P, P], MM_DT, kind="ExternalInput")
    # bf16 output halves the output-DMA tail; the host all-reduce of the
    # two per-batch partials runs in fp32
    outT = nc.dram_tensor("outT", [C, T], MM_DT, kind="ExternalOutput")

    Exp = mybir.ActivationFunctionType.Exp

    with tile.TileContext(nc) as tc:
        with (
            tc.tile_pool(name="persist", bufs=1) as persist,
            tc.tile_pool(name="xtp", bufs=2) as xtp,
            tc.tile_pool(name="ptp", bufs=8) as ptp,
            tc.tile_pool(name="rawp", bufs=3) as rawp,
            tc.tile_pool(name="misc", bufs=2) as misc,
            tc.tile_pool(name="outp", bufs=3) as outp,
            tc.tile_pool(name="ps_s", bufs=2, space="PSUM") as ps_s,
            tc.tile_pool(name="ps_o", bufs=1, space="PSUM") as ps_o,
            tc.tile_pool(name="ps_gen", bufs=2, space="PSUM") as ps_gen,
        ):
            # Q^T / K^T with head pairs stacked on partitions: tile hp holds
            # head 2hp in rows 0-63 and head 2hp+1 in rows 64-127.  Split
            # per head-pair (and vaug per block) so the dependency tracker
            # sees exact producer/consumer pairs instead of one big tensor.
            qts = [persist.tile([P, T], MM_DT, tag=f"qt{i}", name=f"qt{i}") for i in range(NHP)]
            kts = [persist.tile([P, T], MM_DT, tag=f"kt{i}", name=f"kt{i}") for i in range(NHP)]
            vaugs = [
                persist.tile([P, 4, HL, HS + 1], MM_DT, tag=f"vaug{i}", name=f"vaug{i}")
                for i in range(NT)
            ]
            bias_sb = persist.tile([P, C // P], F32)
            wq_sb = persist.tile([P, NCC, HD], MM_DT, tag="wq")
            wk_sb = persist.tile([P, NCC, HD], MM_DT, tag="wk")
            wv_sb = persist.tile([P, NCC, HD], MM_DT, tag="wv")
            wpT_sb = persist.tile([P, HD // P, C], MM_DT, tag="wpT")
            masks_sb = persist.tile([P, 2, P], MM_DT, tag="masks")
            combos = [persist.tile([P, T], MM_DT, tag=f"combo{i}", name=f"combo{i}") for i in range(NHP)]
            ones1 = persist.tile([1, HS], F32, tag="ones1")

            # ones column for the softmax-denominator row of the PV matmul
            for vv in vaugs:
                nc.vector.memset(vv[:, :, :, HS : HS + 1], 1.0)
            nc.vector.memset(ones1[:], 1.0)
            # DMA priority follows first consumption: q-chain 0 (wq + x^T
            # block 0), then the v chains (wv), then k chains (wk);
            # wpT/masks/bias are needed much later.
            xt_tiles = {}
            xt0 = xtp.tile([P, NCC, 512], MM_DT, tag="xt")
            xt_tiles[0] = xt0
            xTr = xT[:].rearrange("(co p) t -> p co t", p=P)
            wqr = wq[:].rearrange("(co p) n -> p co n", p=P)
            wkr = wk[:].rearrange("(co p) n -> p co n", p=P)
            wvr = wv[:].rearrange("(co p) n -> p co n", p=P)
            for _h in range(4):
                nc.sync.dma_start(xt0[:, 2 * _h : 2 * _h + 2, :], xTr[:, 2 * _h : 2 * _h + 2, 0:512])
            for _h in range(4):
                nc.scalar.dma_start(wq_sb[:, 2 * _h : 2 * _h + 2, :], wqr[:, 2 * _h : 2 * _h + 2, :])
            for _h in range(4):
                nc.scalar.dma_start(wv_sb[:, 2 * _h : 2 * _h + 2, :], wvr[:, 2 * _h : 2 * _h + 2, :])
            for _h in range(4):
                nc.sync.dma_start(wk_sb[:, 2 * _h : 2 * _h + 2, :], wkr[:, 2 * _h : 2 * _h + 2, :])
            nc.gpsimd.dma_start(masks_sb[:, 0, :], masks[:])
            nc.gpsimd.dma_start(masks_sb[:, 1, :], masks[:])
            nc.gpsimd.dma_start(bias_sb[:], bias[:].rearrange("(db p) -> p db", p=P))

            def qk_quarter(tb, part):
                """Q and K projection chains for head-pair `part` of
                block tb."""
                tsl = slice(tb * 512, (tb + 1) * 512)
                xt = xt_tiles[tb]
                hsl = slice(part * P, (part + 1) * P)
                for w_sb, dst in ((wq_sb, qts[part]), (wk_sb, kts[part])):
                    pqk = ps_gen.tile([P, 512], F32, tag="pqk")
                    for co in range(NCC):
                        nc.tensor.matmul(
                            pqk[:],
                            w_sb[:, co, hsl],
                            xt[:, co, :],
                            start=(co == 0),
                            stop=(co == NCC - 1),
                        )
                    if tb == 0:
                        # ACT is exp-idle before attention starts; use it
                        # for block 0's psum evacuation
                        nc.scalar.copy(out=dst[:, tsl], in_=pqk[:])
                    else:
                        nc.vector.tensor_copy(out=dst[:, tsl], in_=pqk[:])

            def v_quarter(tb, part):
                """V projection chain for token sub-block `part` of block
                tb.  part=0 also issues the x^T block load."""
                tsl = slice(tb * 512, (tb + 1) * 512)
                if part == 0 and tb not in xt_tiles:
                    xt = xtp.tile([P, NCC, 512], MM_DT, tag="xt")
                    xt_tiles[tb] = xt
                    for _h in range(2):
                        nc.sync.dma_start(
                            xt[:, 4 * _h : 4 * _h + 4, :],
                            xTr[:, 4 * _h : 4 * _h + 4, tsl],
                        )
                xt = xt_tiles[tb]
                pv = ps_gen.tile([P, 512], F32, tag="pqk")
                for co in range(NCC):
                    nc.tensor.matmul(
                        pv[:],
                        xt[:, co, part * P : (part + 1) * P],
                        wv_sb[:, co, :],
                        start=(co == 0),
                        stop=(co == NCC - 1),
                    )
                nc.vector.tensor_copy(
                    out=vaugs[tb][:, part, :, 0:HS],
                    in_=pv[:].rearrange("p (h d) -> p h d", h=HL),
                )

            def attn_pair(qb, hp, fast_tail=False):
                """Causal attention for head pair hp over query block qb."""
                q0 = qb * 512
                nkc = 4 * (qb + 1)
                po = ps_o.tile([P, 2, 512], F32, tag="po")
                for kc in range(nkc):
                    mi = kc - 4 * qb
                    # diagonal chunks: columns q < kc*128-q0 are fully
                    # masked; skip them in scores, exp and PV
                    c0 = kc * P - q0 if mi >= 0 else 0
                    pss = ps_s.tile([P, 2, 512], F32, tag="pss")
                    for j in range(2):
                        r0 = 64 * j
                        nc.tensor.matmul(
                            pss[:, j, c0:512],
                            kts[hp][r0 : r0 + 64, kc * P : (kc + 1) * P],
                            qts[hp][r0 : r0 + 64, q0 + c0 : q0 + 512],
                            start=True,
                            stop=True,
                            tile_position=(r0, 0),
                        )
                    pt = ptp.tile([P, 2, 512], MM_DT, tag="pt")
                    nc.scalar.activation(
                        pt[:, :, c0:512], pss[:, :, c0:512], Exp, scale=EXP_SCALE
                    )
                    if mi >= 0:
                        # only the 128x128 diagonal block needs the
                        # triangle mask (one mul covers both heads)
                        msl = slice(mi * P, (mi + 1) * P)
                        nc.vector.tensor_mul(
                            out=pt[:, :, msl],
                            in0=pt[:, :, msl],
                            in1=masks_sb[:],
                        )
                    for j in range(2):
                        nc.tensor.matmul(
                            po[0 : HS + 1, j, c0:512],
                            vaugs[kc // 4][:, kc % 4, 2 * hp + j, :],
                            pt[:, j, c0:512],
                            start=(kc == 0),
                            stop=(kc == nkc - 1),
                        )
                # single psum->sbuf copy frees po for the next pair FIRST
                # (the next pair's PV needs the banks ~1.4us in); the
                # normalization chain then runs off SBUF
                if fast_tail:
                    # latency-critical final pair: no next pair needs the
                    # po banks, so skip the raw staging copy and run the
                    # chain per head straight off PSUM ([1,N] DVE ops are
                    # lane-serial: ~0.6us for N=512)
                    for j in range(2):
                        dj = misc.tile([1, 2, 512], F32, tag="den2")
                        nc.vector.tensor_copy(out=dj[0:1, 0, :], in_=po[HS : HS + 1, j, :])
                        rj = misc.tile([1, 2, 512], F32, tag="rc")
                        nc.vector.reciprocal_approx_fast(rj[0:1, 0, :], dj[0:1, 0, :])
                        rbj = misc.tile([HS, 2, 512], F32, tag="rb")
                        nc.gpsimd.partition_broadcast(rbj[:, 0, :], rj[0:1, 0, :])
                        nc.vector.tensor_mul(
                            out=combos[hp][j * HS : (j + 1) * HS, q0 : q0 + 512],
                            in0=po[0:HS, j, :],
                            in1=rbj[:, 0, :],
                        )
                else:
                    raw = rawp.tile([HS + 1, 2, 512], MM_DT, tag="raw")
                    nc.vector.tensor_copy(out=raw[:], in_=po[0 : HS + 1, :, :])
                    den2 = misc.tile([1, 2, 512], F32, tag="den2")
                    nc.vector.tensor_copy(out=den2[:], in_=raw[HS : HS + 1, :, :])
                    rc = misc.tile([1, 2, 512], F32, tag="rc")
                    nc.vector.reciprocal_approx_fast(rc[:], den2[:])
                    rb = misc.tile([HS, 2, 512], F32, tag="rb")
                    nc.gpsimd.partition_broadcast(rb[:], rc[:])
                    nc.vector.tensor_mul(
                        out=combos[hp][0:HS, q0 : q0 + 512],
                        in0=raw[0:HS, 0, :],
                        in1=rb[:, 0, :],
                    )
                    nc.vector.tensor_mul(
                        out=combos[hp][HS:P, q0 : q0 + 512],
                        in0=raw[0:HS, 1, :],
                        in1=rb[:, 1, :],
                    )

            def outproj(qb, dbs=None, tail=False):
                q0 = qb * 512
                dma_engs = [nc.sync, nc.gpsimd, nc.scalar]
                for db in dbs if dbs is not None else range(C // P):
                    pp = ps_gen.tile([P, 512], F32, tag="pqk")
                    for co in range(HD // P):
                        nc.tensor.matmul(
                            pp[:],
                            wpT_sb[:, co, db * P : (db + 1) * P],
                            combos[co][:, q0 : q0 + 512],
                            start=(co == 0),
                            stop=(co == HD // P - 1),
                        )
                    ot = outp.tile([P, 512], MM_DT, tag="ot")
                    if tail and db % 2 == 1:
                        # post-exp region: ACT is idle — alternating the
                        # bias-adds between ACT and DVE halves the
                        # psum-slot release latency that paces the chains
                        nc.scalar.add(ot[:], pp[:], bias_sb[:, db : db + 1])
                    else:
                        nc.vector.tensor_scalar_add(ot[:], pp[:], bias_sb[:, db : db + 1])
                    eng = dma_engs[db % 3] if tail else dma_engs[db % 2]
                    eng.dma_start(outT[db * P : (db + 1) * P, q0 : q0 + 512], ot[:])

            # ---- schedule ----
            # The input DMA paces the first ~13us; dummy matmuls on a
            # zeroed SBUF tile keep the PE HAM activity monitor busy so
            # the clock gate is at 8/8 (2.4 GHz) when real matmuls start
            # (otherwise all of QKV(0) runs at the cold 1.2 GHz).
            dummy = persist.tile([P, P], MM_DT, tag="dummy")
            nc.vector.memset(dummy[:], 0.0)
            # trigger the ~2.7us exp ACT-table load during the DMA head
            # instead of at the first real softmax (write to scratch so
            # the warm-up matmuls below don't serialize behind it)
            nc.scalar.activation(
                ones1[0:1, 0:1], dummy[0:1, 0:1], Exp, scale=EXP_SCALE
            )
            pw = ps_gen.tile([P, P], F32, tag="pqk")
            for _w in range(70):
                nc.tensor.matmul(pw[:, 0:64], dummy[:], dummy[:, 0:64], start=True, stop=True)
            # attn(qb, hp) needs: q/k chains (tb<=qb, part=hp) and ALL v
            # chains of blocks <= qb.  Block 0's chains cascade so
            # attention starts as soon as [qk(0,0), v(0,0..3)] land; later
            # blocks' projections sit a full window ahead as exp-gap
            # filler.  qb 3 has no QKV left, so the output projections of
            # blocks 0-2 are delayed into it.
            qk_quarter(0, 0)
            for part in range(4):
                v_quarter(0, part)
            for qb in range(NT):
                for hp in range(NHP):
                    attn_pair(qb, hp, fast_tail=(qb == NT - 1 and hp == NHP - 1))
                    if qb == 0 and hp < 3:
                        qk_quarter(0, hp + 1)
                    if qb == 0 and hp == 3:
                        # wpT is first needed by outproj during the last
                        # window; loading it now keeps it clear of the
                        # startup DMA burst
                        nc.gpsimd.dma_start(wpT_sb[:], wpT[:].rearrange("(co p) n -> p co n", p=P))
                    if qb < NT - 1:
                        v_quarter(qb + 1, hp)
                        qk_quarter(qb + 1, hp)
                    elif hp < 3:
                        outproj(hp, dbs=range(0, 4))
            # the remaining hp3-independent outproj chains flow into the
            # last pair's exp gaps and its normalization-chain latency
            # (keeping the PE warm); outproj(3) runs immediately after
            for qq in range(3):
                outproj(qq, dbs=range(4, 8))
            outproj(NT - 1, tail=True)

    nc.finalize()
    return nc


def _causal_masks():
    # lower-triangle [128,128]: 1.0 iff kl <= ql (applied multiplicatively
    # post-exp to the single diagonal block of each diagonal key chunk)
    kl = np.arange(P)[:, None]
    ql = np.arange(P)[None, :]
    return (kl <= ql).astype(np.float32)


def _in_maps(x, Wq, Wk, Wv, Wproj, bproj):
    masks = _causal_masks()
    zeros_bias = np.zeros_like(bproj)
    maps = []
    for core in range(8):
        b, hg = core // 2, core % 2
        hs = slice(hg * HL, (hg + 1) * HL)
        maps.append(
            {
                "xT": np.ascontiguousarray(x[b].T).astype(MM_NP),
                "wq": np.ascontiguousarray(
                    Wq[hs].transpose(1, 0, 2).reshape(C, HD).astype(MM_NP)
                ),
                "wk": np.ascontiguousarray(
                    Wk[hs].transpose(1, 0, 2).reshape(C, HD).astype(MM_NP)
                ),
                "wv": np.ascontiguousarray(
                    Wv[hs].transpose(1, 0, 2).reshape(C, HD).astype(MM_NP)
                ),
                "wpT": np.ascontiguousarray(Wproj[:, hg * HD : (hg + 1) * HD].T).astype(MM_NP),
                "bias": np.ascontiguousarray(bproj if hg == 0 else zeros_bias),
                "masks": masks.astype(MM_NP),
            }
        )
    return maps


def get_program():
    global _PROGRAM
    if _PROGRAM is None:
        _PROGRAM = _build_program()
    return _PROGRAM


def kernel(x, Wq, Wk, Wv, Wproj, bproj, _run_kwargs=None):
    x = np.asarray(x, dtype=np.float32)
    Wq = np.asarray(Wq, dtype=np.float32)
    Wk = np.asarray(Wk, dtype=np.float32)
    Wv = np.asarray(Wv, dtype=np.float32)
    Wproj = np.asarray(Wproj, dtype=np.float32)
    bproj = np.asarray(bproj, dtype=np.float32)

    nc = get_program()
    res = run_bass_kernel_spmd(
        nc,
        _in_maps(x, Wq, Wk, Wv, Wproj, bproj),
        core_ids=list(range(8)),
        **(_run_kwargs or {}),
    )
    out = np.empty((B, T, C), dtype=np.float32)
    for b in range(B):
        out[b] = (
            res.results[2 * b]["outT"].astype(np.float32)
            + res.results[2 * b + 1]["outT"].astype(np.float32)
        ).T
    kernel.last_results = res
    return out
